# revision 43
# baseline (speedup 1.0000x reference)
"""Trainium2 Bass kernel for nn_Encoder_79096117723504 (gnn_message_passing).

Node-space collapse (see kernel_v0): every edge-level quantity is a gather of
a node-level one, so the [H,F,T] edge attention reduces to 512-node space with
histogram weights.  This version restructures the device program around
measured TRN2 engine costs:

  * fp8e4 DoubleRow matmuls (216 ns per 256-contraction x 512-free MM, 2x the
    fp16 rate) for the score path: q/k projections and all 16 per-head
    [512,512] score matmuls.  Host pre-scales Wq/Wk/Wcb by 64 so fp8 operand
    magnitudes sit in e4m3's normal range; the 1/4096 comes out in the exp
    scale.  The v/ctx/Wd path stays fp16 (fp8 there costs 6-8e-3 rel err).
  * histogram counts ct/cf computed on HOST (np.bincount); ln(ct) is folded
    into the exp bias (per-partition in the S^T layout), which also turns the
    softmax Z row into a plain ones-contraction and drops the ct multiplies.
  * ONE batched [16,512] reciprocal for all 16 heads' softmax normalizers
    (reciprocal is ~3.3 us regardless of partition count), with Z rows
    gathered by tiny SBUF DMAs and 1/Z re-broadcast via a K=2 PE matmul.
  * LayerNorm gain/bias applied on host after pooling (exact: pooling is
    linear), with rstd folded into the pooling weight vector.
  * gpsimd unused (measured 7.5 us per [128,512] op); elementwise work split
    DVE/ACT by measured rates (ts ~330 ns, ACT copy ~686 ns, exp ~573 ns).

Per core one (segment, direction) sub-problem; cores 6,7 duplicate 0,1.
"""
import math
import os
import sys

import numpy as np

for p in ('/opt/trn_rl_repo', '/root/.axon_site/_ro/trn_rl_repo'):
    if os.path.isdir(p) and p not in sys.path:
        sys.path.insert(0, p)

import concourse.bass as bass
import concourse.mybir as mybir
from concourse import bacc, tile
from concourse.bass_utils import run_bass_kernel_spmd

F32 = mybir.dt.float32
F16 = mybir.dt.float16
F8 = mybir.dt.float8e4
AF = mybir.ActivationFunctionType
ALU = mybir.AluOpType
DR = mybir.MatmulPerfMode.DoubleRow

D = 768
H = 16
DH = 48
SEQ = 512
NE = 2048
EPS = 1e-5
SCALE = math.sqrt(D / H)
WS = 64.0                  # host weight pre-scale for the fp8 score path
EXPSC = 1.0 / (WS * WS * SCALE)   # exp scale: S_psum = WS^2 * S_true
KT = 6
VT = 4
PADK = H * 64

_NC_CACHE = {}


def build_nc():
    nc = bacc.Bacc(None, target_bir_lowering=False)

    # -------- DRAM I/O (bound by position) ---------------------------------
    hT8d = nc.declare_dram_parameter("hT8", [D, SEQ], F8, isOutput=False)
    h_nmd = nc.declare_dram_parameter("h_nm", [SEQ, D], F16, isOutput=False)
    wq8d = nc.declare_dram_parameter("Wq8", [D, D], F8, isOutput=False)
    wk8d = nc.declare_dram_parameter("Wk8", [D, D], F8, isOutput=False)
    wcb8d = nc.declare_dram_parameter("Wcb8", [D, H], F8, isOutput=False)
    wv16d = nc.declare_dram_parameter("WvTe", [D + 1, D], F16, isOutput=False)
    wd16d = nc.declare_dram_parameter("WdTe", [PADK + 1, D], F16, isOutput=False)
    mixd = nc.declare_dram_parameter("mixT", [D, H], F32, isOutput=False)
    lnctd = nc.declare_dram_parameter("lnct", [128, VT], F32, isOutput=False)
    cf16d = nc.declare_dram_parameter("cf16", [128, VT], F16, isOutput=False)
    seld = nc.declare_dram_parameter("sel", [40, 8 * 128], F16, isOutput=False)
    o_out = nc.declare_dram_parameter("o", [1, D], F32, isOutput=True)

    with tile.TileContext(nc) as tc:
        with (
            tc.tile_pool(name="const", bufs=1) as cp,
            tc.tile_pool(name="psA", bufs=3, space="PSUM") as psA,
            tc.tile_pool(name="psB", bufs=2, space="PSUM") as psB,
            tc.tile_pool(name="psC", bufs=1, space="PSUM") as psC,
            tc.tile_pool(name="psD", bufs=1, space="PSUM") as psD,
        ):
            # ---------------- bulk loads (priority order: the k/q projection
            # inputs first -- the prologue is HBM-ingress-bound) --------------
            hT8 = cp.tile([128, KT, SEQ], F8, tag="hT8")
            nc.sync.dma_start(hT8[:], hT8d[:, :].rearrange("(t p) u -> p t u", p=128))
            wk8 = cp.tile([128, KT, D], F8, tag="wk8")
            nc.sync.dma_start(wk8[:], wk8d[:, :].rearrange("(t p) d -> p t d", p=128))
            wq8 = cp.tile([128, KT, D], F8, tag="wq8")
            nc.sync.dma_start(wq8[:], wq8d[:, :].rearrange("(t p) d -> p t d", p=128))
            mix = cp.tile([128, KT, H], F32, tag="mix")
            nc.sync.dma_start(mix[:], mixd[:, :].rearrange("(t p) h -> p t h", p=128))
            wcb8 = cp.tile([128, KT, H], F8, tag="wcb8")
            nc.sync.dma_start(wcb8[:], wcb8d[:, :].rearrange("(t p) h -> p t h", p=128))
            lnct = cp.tile([128, VT], F32, tag="lnct")
            nc.sync.dma_start(lnct[:], lnctd[:])
            wv16 = cp.tile([128, KT, D], F16, tag="wv16")
            nc.sync.dma_start(wv16[:], wv16d[0:D, :].rearrange("(t p) d -> p t d", p=128))
            bv = cp.tile([1, D], F16, tag="bv")
            nc.sync.dma_start(bv[:], wv16d[D:D + 1, :])
            wd16 = cp.tile([128, 8, D], F16, tag="wd16")
            nc.sync.dma_start(wd16[:], wd16d[0:PADK, :].rearrange("(t p) d -> p t d", p=128))
            bd = cp.tile([1, D], F16, tag="bd")
            nc.sync.dma_start(bd[:], wd16d[PADK:PADK + 1, :])
            h_nm = cp.tile([128, VT, D], F16, tag="h_nm")
            nc.sync.dma_start(h_nm[:], h_nmd[:, :].rearrange("(t p) d -> p t d", p=128))
            cf16 = cp.tile([128, VT], F16, tag="cf16")
            nc.sync.dma_start(cf16[:], cf16d[:])

            ones = cp.tile([1, SEQ], F16, tag="ones")
            nc.vector.memset(ones[:], 1.0)
            # sel: per-k-tile one-hot pair-selectors; rb[m,u] = r of head
            # (2kk + (m>=64)) read straight out of rall's aligned rows
            sel = cp.tile([40, 8, 128], F16, tag="sel")
            nc.sync.dma_start(sel[:], seld[:, :].rearrange("r (k m) -> r k m", m=128))

            # PE warm-up: dummy matmuls while the bulk DMAs land, so the HAM
            # clock gate reaches K=8/8 before the first real matmul.
            warm = psD.tile([128, SEQ], F32, tag="rb", name="warm")
            for i in range(13):
                nc.tensor.matmul(warm[:], ones[:, 0:128], ones[:, :],
                                 start=True, stop=True)

            # ---------------- k/q projections (fp8 DoubleRow) ----------------
            # k first: kT16 heads the longest dependency chain (kmT -> scores)
            qT8 = cp.tile([128, KT, SEQ], F8, tag="qT8")
            kT16 = cp.tile([128, KT, SEQ], F16, tag="kT16")
            for m in range(KT):
                ps = psA.tile([128, SEQ], F32, tag="psA")
                for kp in range(3):
                    nc.tensor.matmul(ps[:], wk8[:, 2 * kp:2 * kp + 2, m * 128:(m + 1) * 128],
                                     hT8[:, 2 * kp:2 * kp + 2, :],
                                     start=(kp == 0), stop=(kp == 2), perf_mode=DR)
                nc.vector.tensor_copy(kT16[:, m, :], ps[:])
            for m in range(KT):
                ps = psA.tile([128, SEQ], F32, tag="psA")
                for kp in range(3):
                    nc.tensor.matmul(ps[:], wq8[:, 2 * kp:2 * kp + 2, m * 128:(m + 1) * 128],
                                     hT8[:, 2 * kp:2 * kp + 2, :],
                                     start=(kp == 0), stop=(kp == 2), perf_mode=DR)
                nc.scalar.activation(qT8[:, m, :], ps[:], AF.Copy)

            # ---------------- content bias + ln(ct) fold ----------------
            # cbs[v,t,h] = cb[v,h]/SCALE + ln(ct[v]); per-partition exp bias.
            cbs = cp.tile([128, VT, H], F32, tag="cbs")
            for t in range(VT):
                ps = psA.tile([128, H], F32, tag="psA", name=f"cb{t}")
                for k in range(KT):
                    nc.tensor.matmul(ps[:], hT8[:, k, t * 128:(t + 1) * 128],
                                     wcb8[:, k, :], start=(k == 0), stop=(k == KT - 1))
                nc.vector.tensor_scalar(cbs[:, t, :], ps[:], 1.0 / (WS * SCALE),
                                        lnct[:, t:t + 1],
                                        op0=ALU.mult, op1=ALU.add)

            # ---------------- v (fp16), padded head-block layout -------------
            # 64-block per head: [v 0:32 | Z-slot=1 | v 32:48 | 15 zeros]; the
            # ones in the Z slot make psc row 32 the softmax normalizer Z
            # (ct is folded into E via the ln(ct) exp bias).
            vct = cp.tile([128, VT, PADK], F8, tag="vct")
            for t in range(VT):
                blk0 = vct[:, t, :].rearrange("p (h c) -> p h c", c=64)
                nc.vector.memset(blk0[:, :, 49:64], 0.0)   # pad cols only
                nc.vector.memset(blk0[:, :, 32:33], 1.0)   # softmax-Z ones slot
                for ns, (c0, c1) in enumerate(((0, 384), (384, 768))):
                    ps = psA.tile([128, 384], F32, tag="psA")
                    for k in range(KT):
                        nc.tensor.matmul(ps[:], hT8[:, k, t * 128:(t + 1) * 128],
                                         wv16[:, k, c0:c1], start=(k == 0), stop=False)
                    nc.tensor.matmul(ps[:], ones[:, t * 128:(t + 1) * 128],
                                     bv[:, c0:c1], start=False, stop=True)
                    blk = vct[:, t, 64 * 8 * ns:64 * 8 * (ns + 1)].rearrange(
                        "p (h c) -> p h c", c=64)
                    src = ps[:, :].rearrange("p (h c) -> p h c", c=DH)
                    nc.vector.tensor_copy(blk[:, :, 0:32], src[:, :, 0:32])
                    nc.vector.tensor_copy(blk[:, :, 33:49], src[:, :, 32:DH])

            # ---------------- attention heads ----------------
            ctxU = cp.tile([128, 8, SEQ], F16, tag="ctxU")   # unnormalized ctx
            ctxT = cp.tile([128, 8, SEQ], F16, tag="ctxT")   # normalized
            # Z rows: heads 0-7 at partitions 0-7, heads 8-15 at 32-39 (engine
            # ops need 32-aligned partition bases per batch)
            Zall = cp.tile([40, SEQ], F16, tag="Zall")
            rall = cp.tile([40, SEQ], F16, tag="rall")

            def zrow(hh):
                return hh if hh < 8 else 24 + hh

            def normalize_half(nb):
                """Batch-normalize head pairs 4*nb..4*nb+3: one batched
                [8,512] DVE reciprocal (cost is free-dim-bound, so batching is
                8x cheaper than per-head), then per k-tile a K=8 one-hot
                matmul replicates the two heads' 1/Z rows across the 128
                partitions, and an f16 multiply normalizes ctx."""
                r0 = 32 * nb
                with nc.allow_low_precision(reason="softmax 1/Z in f16"):
                    nc.vector.reciprocal(rall[r0:r0 + 8, :], Zall[r0:r0 + 8, :])
                for kk in range(4 * nb, 4 * nb + 4):
                    rb = psD.tile([128, SEQ], F32, tag="rb", name=f"rb{kk}")
                    nc.tensor.matmul(rb[:], sel[r0:r0 + 8, kk, :],
                                     rall[r0:r0 + 8, :], start=True, stop=True)
                    nc.vector.tensor_mul(ctxT[:, kk, :], ctxU[:, kk, :], rb[:])

            with tc.tile_pool(name="head", bufs=2) as hp:
                for hpi in range(8):
                    # DoubleRow matmuls need dst partition base 0: one [64,512]
                    # PSUM tile per head (separate banks), cast into the packed
                    # [128,...] ctxU layout afterwards.
                    pscs = [psB.tile([64, SEQ], F32, tag="psc_e", bufs=1,
                                     name=f"psce{hpi}"),
                            psB.tile([64, SEQ], F32, tag="psc_o", bufs=1,
                                     name=f"psco{hpi}")]
                    Es = []
                    for sub in range(2):
                        hh = 2 * hpi + sub
                        kmT = hp.tile([128, KT, SEQ], F8, tag="kmT", bufs=3)
                        for k in range(KT):
                            nc.vector.tensor_scalar(kmT[:, k, :], kT16[:, k, :],
                                                    mix[:, k, hh:hh + 1], None,
                                                    op0=ALU.mult)
                        E8 = hp.tile([128, VT, SEQ], F8, tag="E", bufs=3)
                        for t in range(VT):
                            ps = psA.tile([128, SEQ], F32, tag="psA")
                            for kp in range(3):
                                nc.tensor.matmul(
                                    ps[:], kmT[:, 2 * kp:2 * kp + 2, t * 128:(t + 1) * 128],
                                    qT8[:, 2 * kp:2 * kp + 2, :],
                                    start=(kp == 0), stop=(kp == 2), perf_mode=DR)
                            nc.scalar.activation(E8[:, t, :], ps[:], AF.Exp,
                                                 bias=cbs[:, t, hh:hh + 1], scale=EXPSC)
                        Es.append(E8)
                    if hpi == 4:
                        normalize_half(0)   # overlaps pairs 5-7
                    # ctx matmuls after BOTH subs' scores: exp latency hidden
                    for sub in range(2):
                        hh = 2 * hpi + sub
                        for tp in range(2):
                            nc.tensor.matmul(pscs[sub][:, :],
                                             vct[:, 2 * tp:2 * tp + 2, hh * 64:(hh + 1) * 64],
                                             Es[sub][:, 2 * tp:2 * tp + 2, :],
                                             start=(tp == 0), stop=(tp == 1),
                                             perf_mode=DR)
                    # copy Z rows (PSUM row 32 of each sub) to aligned slots;
                    # single-row DVE/ACT ops are free-dim-bound (~0.5us each),
                    # so copy them as part of the full psc casts instead:
                    nc.vector.tensor_copy(ctxU[0:64, hpi, :], pscs[0][:])
                    nc.scalar.activation(ctxU[64:128, hpi, :], pscs[1][:], AF.Copy)
                    za, zb = zrow(2 * hpi), zrow(2 * hpi + 1)
                    nc.sync.dma_start(Zall[za:za + 1, :], ctxU[32:33, hpi, :])
                    nc.sync.dma_start(Zall[zb:zb + 1, :], ctxU[96:97, hpi, :])
            # Early Wd partials over the already-normalized k-tiles 0-3, plus
            # dummy matmuls gated on the last pair's ctx, keep the PE busy
            # (and the HAM clock-gate warm) through the serial tail
            # normalization chain for heads 8-15.
            wd_open = {}
            with tc.high_priority():
                for i in range(10):
                    nc.tensor.matmul(warm[:], ctxU[:, 7, 0:128], ctxU[:, 7, :],
                                     start=True, stop=True)
                for (ut, ns, c0, c1) in ((0, 0, 0, 512), (0, 1, 512, 768),
                                         (1, 0, 0, 512)):
                    ps = psA.tile([128, c1 - c0], F32, tag="psA", name=f"wde{ut}{ns}")
                    for k in range(4):
                        nc.tensor.matmul(ps[:], ctxT[:, k, ut * 128:(ut + 1) * 128],
                                         wd16[:, k, c0:c1], start=(k == 0), stop=False)
                    wd_open[(ut, ns)] = ps

            normalize_half(1)

            # ---------------- epilogue: Wd, residual, LN, pooled mean --------
            o_ps = [psC.tile([1, 512], F32, tag="ops0", name="ops0"),
                    psC.tile([1, 256], F32, tag="ops1", name="ops1")]
            with tc.tile_pool(name="epi", bufs=2) as ep:
                for ut in range(VT):
                    x = ep.tile([128, D], F32, tag="x")
                    for ns, (c0, c1) in enumerate(((0, 512), (512, 768))):
                        ps = wd_open.pop((ut, ns), None)
                        if ps is None:
                            ps = psA.tile([128, c1 - c0], F32, tag="psA")
                            k0 = 0
                        else:
                            k0 = 4
                        for k in range(k0, 8):
                            nc.tensor.matmul(ps[:], ctxT[:, k, ut * 128:(ut + 1) * 128],
                                             wd16[:, k, c0:c1], start=(k == 0), stop=False)
                        nc.tensor.matmul(ps[:], ones[:, ut * 128:(ut + 1) * 128],
                                         bd[:, c0:c1], start=False, stop=True)
                        nc.vector.tensor_add(x[:, c0:c1], ps[:], h_nm[:, ut, c0:c1])
                    negmu = ep.tile([128, 1], F32, tag="negmu")
                    nc.vector.reduce_sum(negmu[:], x[:], axis=mybir.AxisListType.X)
                    nc.vector.tensor_scalar(negmu[:], negmu[:], -1.0 / D, None,
                                            op0=ALU.mult)
                    sq = ep.tile([128, D], F16, tag="sq")
                    ssq = ep.tile([128, 1], F32, tag="ssq")
                    nc.scalar.activation(sq[:], x[:], AF.Square,
                                         bias=negmu[:], scale=1.0, accum_out=ssq[:])
                    nc.vector.tensor_scalar(ssq[:], ssq[:], 1.0 / D, EPS,
                                            op0=ALU.mult, op1=ALU.add)
                    nc.scalar.sqrt(ssq[:], ssq[:])
                    rstd = ep.tile([128, 1], F32, tag="rstd")
                    nc.vector.reciprocal(rstd[:], ssq[:])
                    xc16 = ep.tile([128, D], F16, tag="xc16")
                    nc.vector.tensor_scalar(xc16[:], x[:], negmu[:], None,
                                            op0=ALU.add)
                    w16 = ep.tile([128, 1], F16, tag="w16")
                    nc.vector.tensor_scalar(w16[:], cf16[:, ut:ut + 1], rstd[:],
                                            None, op0=ALU.mult)
                    for ns, (c0, c1) in enumerate(((0, 512), (512, 768))):
                        nc.tensor.matmul(o_ps[ns][:], w16[:], xc16[:, c0:c1],
                                         start=(ut == 0), stop=(ut == VT - 1))

            o_sb = cp.tile([1, D], F32, tag="o_sb")
            nc.vector.tensor_copy(o_sb[:, 0:512], o_ps[0][:])
            nc.scalar.activation(o_sb[:, 512:768], o_ps[1][:], AF.Copy)
            nc.sync.dma_start(o_out[:], o_sb[:])

    nc.finalize()
    return nc


def _pad_wd(Wd, bd):
    """[PADK+1, 768] fp16, rows permuted to the ctx block layout
    [dims 0:32 | Z slot | dims 32:48 | 15 pad] per 64-row head block; Z-slot
    and pad rows zero.  Row PADK = bd."""
    WdT = np.asarray(Wd, np.float32).T
    out = np.zeros((PADK + 1, D), np.float16)
    for h in range(H):
        out[h * 64:h * 64 + 32, :] = WdT[h * DH:h * DH + 32, :]
        out[h * 64 + 33:h * 64 + 49, :] = WdT[h * DH + 32:(h + 1) * DH, :]
    out[PADK, :] = np.asarray(bd, np.float32)
    return out


def _core_inputs(h_b, fr, to, W):
    f16, f32 = np.float16, np.float32
    f8 = mybir.dt.np(F8)
    h = np.asarray(h_b, f32)
    hT = np.ascontiguousarray(h.T)
    fr = np.asarray(fr, np.int64) % SEQ
    to = np.asarray(to, np.int64) % SEQ
    ct = np.bincount(to, minlength=SEQ).astype(f32)
    cf = np.bincount(fr, minlength=SEQ).astype(f32)
    lnct = np.where(ct > 0, np.log(np.maximum(ct, 1e-6)), -30.0).astype(f32)
    return {
        "hT8": hT.astype(f8),
        "h_nm": np.ascontiguousarray(h).astype(f16),
        "Wq8": np.ascontiguousarray(np.asarray(W["Wq"], f32).T * WS).astype(f8),
        "Wk8": np.ascontiguousarray(np.asarray(W["Wk"], f32).T * WS).astype(f8),
        "Wcb8": np.ascontiguousarray(np.asarray(W["Wcb"], f32).T * WS).astype(f8),
        "WvTe": np.concatenate(
            [np.asarray(W["Wv"], f32).T, np.asarray(W["bv"], f32)[None, :]],
            axis=0).astype(f16),
        "WdTe": _pad_wd(W["Wd"], W["bd"]),
        "mixT": np.ascontiguousarray(np.asarray(W["mix"], f32).T),
        "lnct": np.ascontiguousarray(lnct.reshape(VT, 128).T),
        "cf16": np.ascontiguousarray(cf.reshape(VT, 128).T).astype(f16),
        "sel": _SEL,
    }


def _mk_sel():
    s = np.zeros((40, 8, 128), np.float16)
    for kk in range(8):
        base, kl = (0, kk) if kk < 4 else (32, kk - 4)
        s[base + 2 * kl, kk, 0:64] = 1.0
        s[base + 2 * kl + 1, kk, 64:128] = 1.0
    return np.ascontiguousarray(s.reshape(40, 8 * 128))


_SEL = _mk_sel()


def kernel(**inputs):
    hs = np.asarray(inputs["hidden_states"], dtype=np.float32)
    fpos = np.asarray(inputs["fpos"], dtype=np.int32)
    tpos = np.asarray(inputs["tpos"], dtype=np.int32)
    Wsets = {}
    for p in ("qtoc", "ctoq"):
        Wsets[p] = {n: np.asarray(inputs[p + "_" + n]) for n in
                    ("Wq", "Wk", "Wcb", "Wv", "Wd", "mix", "bv", "bd", "lng", "lnb")}

    tasks = [(b, d) for b in range(3) for d in ("qtoc", "ctoq")]
    tasks += [tasks[0], tasks[1]]
    in_maps = []
    for (b, d) in tasks:
        fr, to = (fpos[b], tpos[b]) if d == "qtoc" else (tpos[b], fpos[b])
        in_maps.append(_core_inputs(hs[b], fr, to, Wsets[d]))

    if "nc" not in _NC_CACHE:
        _NC_CACHE["nc"] = build_nc()
    nc = _NC_CACHE["nc"]
    res = run_bass_kernel_spmd(nc, in_maps, list(range(8)))
    results = res.results

    out = np.empty((3, 2 * D), np.float32)
    for c, (b, d) in enumerate(tasks[:6]):
        W = Wsets[d]
        p = results[c]["o"].reshape(D).astype(np.float32)
        y = np.asarray(W["lng"], np.float32) * (p / NE) + np.asarray(W["lnb"], np.float32)
        half = 0 if d == "qtoc" else 1
        out[b, half * D:(half + 1) * D] = y
    return out


if __name__ == "__main__":
    import reference
    inp = reference.setup_inputs()
    got = kernel(**{k: np.asarray(v) for k, v in inp.items()})
    exp = np.asarray(reference.reference(**inp))
    print("rel err:", np.abs(got - exp).max() / np.abs(exp).max())


# revision 44
# speedup vs baseline: 1.0025x; 1.0025x over previous
"""Trainium2 Bass kernel for nn_Encoder_79096117723504 (gnn_message_passing).

Node-space collapse (see kernel_v0): every edge-level quantity is a gather of
a node-level one, so the [H,F,T] edge attention reduces to 512-node space with
histogram weights.  This version restructures the device program around
measured TRN2 engine costs:

  * fp8e4 DoubleRow matmuls (216 ns per 256-contraction x 512-free MM, 2x the
    fp16 rate) for the score path: q/k projections and all 16 per-head
    [512,512] score matmuls.  Host pre-scales Wq/Wk/Wcb by 64 so fp8 operand
    magnitudes sit in e4m3's normal range; the 1/4096 comes out in the exp
    scale.  The v/ctx/Wd path stays fp16 (fp8 there costs 6-8e-3 rel err).
  * histogram counts ct/cf computed on HOST (np.bincount); ln(ct) is folded
    into the exp bias (per-partition in the S^T layout), which also turns the
    softmax Z row into a plain ones-contraction and drops the ct multiplies.
  * ONE batched [16,512] reciprocal for all 16 heads' softmax normalizers
    (reciprocal is ~3.3 us regardless of partition count), with Z rows
    gathered by tiny SBUF DMAs and 1/Z re-broadcast via a K=2 PE matmul.
  * LayerNorm gain/bias applied on host after pooling (exact: pooling is
    linear), with rstd folded into the pooling weight vector.
  * gpsimd unused (measured 7.5 us per [128,512] op); elementwise work split
    DVE/ACT by measured rates (ts ~330 ns, ACT copy ~686 ns, exp ~573 ns).

Per core one (segment, direction) sub-problem; cores 6,7 duplicate 0,1.
"""
import math
import os
import sys

import numpy as np

for p in ('/opt/trn_rl_repo', '/root/.axon_site/_ro/trn_rl_repo'):
    if os.path.isdir(p) and p not in sys.path:
        sys.path.insert(0, p)

import concourse.bass as bass
import concourse.mybir as mybir
from concourse import bacc, tile
from concourse.bass_utils import run_bass_kernel_spmd

F32 = mybir.dt.float32
F16 = mybir.dt.float16
F8 = mybir.dt.float8e4
AF = mybir.ActivationFunctionType
ALU = mybir.AluOpType
DR = mybir.MatmulPerfMode.DoubleRow

D = 768
H = 16
DH = 48
SEQ = 512
NE = 2048
EPS = 1e-5
SCALE = math.sqrt(D / H)
WS = 64.0                  # host weight pre-scale for the fp8 score path
EXPSC = 1.0 / (WS * WS * SCALE)   # exp scale: S_psum = WS^2 * S_true
KT = 6
VT = 4
PADK = H * 64

_NC_CACHE = {}


def build_nc():
    nc = bacc.Bacc(None, target_bir_lowering=False)

    # -------- DRAM I/O (bound by position) ---------------------------------
    hT8d = nc.declare_dram_parameter("hT8", [D, SEQ], F8, isOutput=False)
    h_nmd = nc.declare_dram_parameter("h_nm", [SEQ, D], F16, isOutput=False)
    wq8d = nc.declare_dram_parameter("Wq8", [D, D], F8, isOutput=False)
    wk8d = nc.declare_dram_parameter("Wk8", [D, D], F8, isOutput=False)
    wcb8d = nc.declare_dram_parameter("Wcb8", [D, H], F8, isOutput=False)
    wv16d = nc.declare_dram_parameter("WvTe", [D + 1, D], F16, isOutput=False)
    wd16d = nc.declare_dram_parameter("WdTe", [PADK + 1, D], F16, isOutput=False)
    mixd = nc.declare_dram_parameter("mixT", [D, H], F32, isOutput=False)
    lnctd = nc.declare_dram_parameter("lnct", [128, VT], F32, isOutput=False)
    cf16d = nc.declare_dram_parameter("cf16", [128, VT], F16, isOutput=False)
    seld = nc.declare_dram_parameter("sel", [40, 8 * 128], F16, isOutput=False)
    o_out = nc.declare_dram_parameter("o", [1, D], F32, isOutput=True)

    with tile.TileContext(nc) as tc:
        with (
            tc.tile_pool(name="const", bufs=1) as cp,
            tc.tile_pool(name="psA", bufs=3, space="PSUM") as psA,
            tc.tile_pool(name="psB", bufs=2, space="PSUM") as psB,
            tc.tile_pool(name="psC", bufs=1, space="PSUM") as psC,
            tc.tile_pool(name="psD", bufs=1, space="PSUM") as psD,
        ):
            # ---------------- bulk loads (priority order: the k/q projection
            # inputs first -- the prologue is HBM-ingress-bound) --------------
            hT8 = cp.tile([128, KT, SEQ], F8, tag="hT8")
            nc.sync.dma_start(hT8[:], hT8d[:, :].rearrange("(t p) u -> p t u", p=128))
            wk8 = cp.tile([128, KT, D], F8, tag="wk8")
            nc.sync.dma_start(wk8[:], wk8d[:, :].rearrange("(t p) d -> p t d", p=128))
            wq8 = cp.tile([128, KT, D], F8, tag="wq8")
            nc.sync.dma_start(wq8[:], wq8d[:, :].rearrange("(t p) d -> p t d", p=128))
            mix = cp.tile([128, KT, H], F32, tag="mix")
            nc.sync.dma_start(mix[:], mixd[:, :].rearrange("(t p) h -> p t h", p=128))
            wcb8 = cp.tile([128, KT, H], F8, tag="wcb8")
            nc.sync.dma_start(wcb8[:], wcb8d[:, :].rearrange("(t p) h -> p t h", p=128))
            lnct = cp.tile([128, VT], F32, tag="lnct")
            nc.sync.dma_start(lnct[:], lnctd[:])
            wv16 = cp.tile([128, KT, D], F16, tag="wv16")
            nc.sync.dma_start(wv16[:], wv16d[0:D, :].rearrange("(t p) d -> p t d", p=128))
            bv = cp.tile([1, D], F16, tag="bv")
            nc.sync.dma_start(bv[:], wv16d[D:D + 1, :])
            wd16 = cp.tile([128, 8, D], F16, tag="wd16")
            nc.sync.dma_start(wd16[:], wd16d[0:PADK, :].rearrange("(t p) d -> p t d", p=128))
            bd = cp.tile([1, D], F16, tag="bd")
            nc.sync.dma_start(bd[:], wd16d[PADK:PADK + 1, :])
            h_nm = cp.tile([128, VT, D], F16, tag="h_nm")
            nc.sync.dma_start(h_nm[:], h_nmd[:, :].rearrange("(t p) d -> p t d", p=128))
            cf16 = cp.tile([128, VT], F16, tag="cf16")
            nc.sync.dma_start(cf16[:], cf16d[:])

            ones = cp.tile([1, SEQ], F16, tag="ones")
            nc.vector.memset(ones[:], 1.0)
            # sel: per-k-tile one-hot pair-selectors; rb[m,u] = r of head
            # (2kk + (m>=64)) read straight out of rall's aligned rows
            sel = cp.tile([40, 8, 128], F16, tag="sel")
            nc.sync.dma_start(sel[:], seld[:, :].rearrange("r (k m) -> r k m", m=128))

            # PE warm-up: dummy matmuls while the bulk DMAs land, so the HAM
            # clock gate reaches K=8/8 before the first real matmul.
            warm = psD.tile([128, SEQ], F32, tag="rb", name="warm")
            for i in range(13):
                nc.tensor.matmul(warm[:], ones[:, 0:128], ones[:, :],
                                 start=True, stop=True)

            # ---------------- k/q projections (fp8 DoubleRow) ----------------
            # k first: kT16 heads the longest dependency chain (kmT -> scores)
            qT8 = cp.tile([128, KT, SEQ], F8, tag="qT8")
            kT16 = cp.tile([128, KT, SEQ], F16, tag="kT16")
            for m in range(KT):
                ps = psA.tile([128, SEQ], F32, tag="psA")
                for kp in range(3):
                    nc.tensor.matmul(ps[:], wk8[:, 2 * kp:2 * kp + 2, m * 128:(m + 1) * 128],
                                     hT8[:, 2 * kp:2 * kp + 2, :],
                                     start=(kp == 0), stop=(kp == 2), perf_mode=DR)
                nc.vector.tensor_copy(kT16[:, m, :], ps[:])
            for m in range(KT):
                ps = psA.tile([128, SEQ], F32, tag="psA")
                for kp in range(3):
                    nc.tensor.matmul(ps[:], wq8[:, 2 * kp:2 * kp + 2, m * 128:(m + 1) * 128],
                                     hT8[:, 2 * kp:2 * kp + 2, :],
                                     start=(kp == 0), stop=(kp == 2), perf_mode=DR)
                nc.scalar.activation(qT8[:, m, :], ps[:], AF.Copy)

            # ---------------- content bias + ln(ct) fold ----------------
            # cbs[v,t,h] = cb[v,h]/SCALE + ln(ct[v]); per-partition exp bias.
            cbs = cp.tile([128, VT, H], F32, tag="cbs")
            for t in range(VT):
                ps = psA.tile([128, H], F32, tag="psA", name=f"cb{t}")
                for k in range(KT):
                    nc.tensor.matmul(ps[:], hT8[:, k, t * 128:(t + 1) * 128],
                                     wcb8[:, k, :], start=(k == 0), stop=(k == KT - 1))
                nc.vector.tensor_scalar(cbs[:, t, :], ps[:], 1.0 / (WS * SCALE),
                                        lnct[:, t:t + 1],
                                        op0=ALU.mult, op1=ALU.add)

            # ---------------- v (fp16), padded head-block layout -------------
            # 64-block per head: [v 0:32 | Z-slot=1 | v 32:48 | 15 zeros]; the
            # ones in the Z slot make psc row 32 the softmax normalizer Z
            # (ct is folded into E via the ln(ct) exp bias).
            vct = cp.tile([128, VT, PADK], F8, tag="vct")
            for t in range(VT):
                blk0 = vct[:, t, :].rearrange("p (h c) -> p h c", c=64)
                nc.vector.memset(blk0[:, :, 49:64], 0.0)   # pad cols only
                nc.vector.memset(blk0[:, :, 32:33], 1.0)   # softmax-Z ones slot
                for ns, (c0, c1) in enumerate(((0, 384), (384, 768))):
                    ps = psA.tile([128, 384], F32, tag="psA")
                    for k in range(KT):
                        nc.tensor.matmul(ps[:], hT8[:, k, t * 128:(t + 1) * 128],
                                         wv16[:, k, c0:c1], start=(k == 0), stop=False)
                    nc.tensor.matmul(ps[:], ones[:, t * 128:(t + 1) * 128],
                                     bv[:, c0:c1], start=False, stop=True)
                    blk = vct[:, t, 64 * 8 * ns:64 * 8 * (ns + 1)].rearrange(
                        "p (h c) -> p h c", c=64)
                    src = ps[:, :].rearrange("p (h c) -> p h c", c=DH)
                    nc.vector.tensor_copy(blk[:, :, 0:32], src[:, :, 0:32])
                    nc.vector.tensor_copy(blk[:, :, 33:49], src[:, :, 32:DH])

            # ---------------- attention heads ----------------
            ctxU = cp.tile([128, 8, SEQ], F16, tag="ctxU")   # unnormalized ctx
            ctxT = cp.tile([128, 8, SEQ], F16, tag="ctxT")   # normalized
            # Z rows: heads 0-7 at partitions 0-7, heads 8-15 at 32-39 (engine
            # ops need 32-aligned partition bases per batch)
            Zall = cp.tile([40, SEQ], F16, tag="Zall")
            rall = cp.tile([40, SEQ], F16, tag="rall")

            def zrow(hh):
                return hh if hh < 8 else 24 + hh

            def normalize_half(nb):
                """Batch-normalize head pairs 4*nb..4*nb+3: one batched
                [8,512] DVE reciprocal (cost is free-dim-bound, so batching is
                8x cheaper than per-head), then per k-tile a K=8 one-hot
                matmul replicates the two heads' 1/Z rows across the 128
                partitions, and an f16 multiply normalizes ctx."""
                r0 = 32 * nb
                with nc.allow_low_precision(reason="softmax 1/Z in f16"):
                    nc.vector.reciprocal(rall[r0:r0 + 8, :], Zall[r0:r0 + 8, :])
                for kk in range(4 * nb, 4 * nb + 4):
                    rb = psD.tile([128, SEQ], F32, tag="rb", name=f"rb{kk}")
                    nc.tensor.matmul(rb[:], sel[r0:r0 + 8, kk, :],
                                     rall[r0:r0 + 8, :], start=True, stop=True)
                    nc.vector.tensor_mul(ctxT[:, kk, :], ctxU[:, kk, :], rb[:])

            with tc.tile_pool(name="head", bufs=2) as hp:
                for hpi in range(8):
                    # DoubleRow matmuls need dst partition base 0: one [64,512]
                    # PSUM tile per head (separate banks), cast into the packed
                    # [128,...] ctxU layout afterwards.
                    pscs = [psB.tile([64, SEQ], F32, tag="psc_e", bufs=1,
                                     name=f"psce{hpi}"),
                            psB.tile([64, SEQ], F32, tag="psc_o", bufs=1,
                                     name=f"psco{hpi}")]
                    Es = []
                    for sub in range(2):
                        hh = 2 * hpi + sub
                        kmT = hp.tile([128, KT, SEQ], F8, tag="kmT", bufs=3)
                        for k in range(KT):
                            nc.vector.tensor_scalar(kmT[:, k, :], kT16[:, k, :],
                                                    mix[:, k, hh:hh + 1], None,
                                                    op0=ALU.mult)
                        E8 = hp.tile([128, VT, SEQ], F8, tag="E", bufs=3)
                        for t in range(VT):
                            ps = psA.tile([128, SEQ], F32, tag="psA")
                            for kp in range(3):
                                nc.tensor.matmul(
                                    ps[:], kmT[:, 2 * kp:2 * kp + 2, t * 128:(t + 1) * 128],
                                    qT8[:, 2 * kp:2 * kp + 2, :],
                                    start=(kp == 0), stop=(kp == 2), perf_mode=DR)
                            nc.scalar.activation(E8[:, t, :], ps[:], AF.Exp,
                                                 bias=cbs[:, t, hh:hh + 1], scale=EXPSC)
                        Es.append(E8)
                    if hpi == 4:
                        normalize_half(0)   # overlaps pairs 5-7
                    # ctx matmuls after BOTH subs' scores: exp latency hidden
                    for sub in range(2):
                        hh = 2 * hpi + sub
                        for tp in range(2):
                            nc.tensor.matmul(pscs[sub][:, :],
                                             vct[:, 2 * tp:2 * tp + 2, hh * 64:(hh + 1) * 64],
                                             Es[sub][:, 2 * tp:2 * tp + 2, :],
                                             start=(tp == 0), stop=(tp == 1),
                                             perf_mode=DR)
                    # copy Z rows (PSUM row 32 of each sub) to aligned slots;
                    # single-row DVE/ACT ops are free-dim-bound (~0.5us each),
                    # so copy them as part of the full psc casts instead:
                    nc.vector.tensor_copy(ctxU[0:64, hpi, :], pscs[0][:])
                    nc.scalar.activation(ctxU[64:128, hpi, :], pscs[1][:], AF.Copy)
                    za, zb = zrow(2 * hpi), zrow(2 * hpi + 1)
                    nc.sync.dma_start(Zall[za:za + 1, :], ctxU[32:33, hpi, :])
                    nc.sync.dma_start(Zall[zb:zb + 1, :], ctxU[96:97, hpi, :])
            # Early Wd partials over the already-normalized k-tiles 0-3, plus
            # dummy matmuls gated on the last pair's ctx, keep the PE busy
            # (and the HAM clock-gate warm) through the serial tail
            # normalization chain for heads 8-15.
            wd_open = {}
            # Dependency-free warm matmuls: the list scheduler must place them
            # right after the last pair (they are always "ready"), and the
            # in-order PE queue can't run them earlier -- so they bridge the
            # serial tail-normalization chain and keep the HAM clock at 8/8.
            for i in range(14):
                nc.tensor.matmul(warm[:], ones[:, 0:128], ones[:, :],
                                 start=True, stop=True)
            for (ut, ns, c0, c1) in ((0, 0, 0, 512), (0, 1, 512, 768),
                                     (1, 0, 0, 512)):
                ps = psA.tile([128, c1 - c0], F32, tag="psA", name=f"wde{ut}{ns}")
                for k in range(4):
                    nc.tensor.matmul(ps[:], ctxT[:, k, ut * 128:(ut + 1) * 128],
                                     wd16[:, k, c0:c1], start=(k == 0), stop=False)
                wd_open[(ut, ns)] = ps

            normalize_half(1)

            # ---------------- epilogue: Wd, residual, LN, pooled mean --------
            o_ps = [psC.tile([1, 512], F32, tag="ops0", name="ops0"),
                    psC.tile([1, 256], F32, tag="ops1", name="ops1")]
            with tc.tile_pool(name="epi", bufs=2) as ep:
                for ut in range(VT):
                    x = ep.tile([128, D], F32, tag="x")
                    for ns, (c0, c1) in enumerate(((0, 512), (512, 768))):
                        ps = wd_open.pop((ut, ns), None)
                        if ps is None:
                            ps = psA.tile([128, c1 - c0], F32, tag="psA")
                            k0 = 0
                        else:
                            k0 = 4
                        for k in range(k0, 8):
                            nc.tensor.matmul(ps[:], ctxT[:, k, ut * 128:(ut + 1) * 128],
                                             wd16[:, k, c0:c1], start=(k == 0), stop=False)
                        nc.tensor.matmul(ps[:], ones[:, ut * 128:(ut + 1) * 128],
                                         bd[:, c0:c1], start=False, stop=True)
                        nc.vector.tensor_add(x[:, c0:c1], ps[:], h_nm[:, ut, c0:c1])
                    negmu = ep.tile([128, 1], F32, tag="negmu")
                    nc.vector.reduce_sum(negmu[:], x[:], axis=mybir.AxisListType.X)
                    nc.vector.tensor_scalar(negmu[:], negmu[:], -1.0 / D, None,
                                            op0=ALU.mult)
                    sq = ep.tile([128, D], F16, tag="sq")
                    ssq = ep.tile([128, 1], F32, tag="ssq")
                    nc.scalar.activation(sq[:], x[:], AF.Square,
                                         bias=negmu[:], scale=1.0, accum_out=ssq[:])
                    nc.vector.tensor_scalar(ssq[:], ssq[:], 1.0 / D, EPS,
                                            op0=ALU.mult, op1=ALU.add)
                    nc.scalar.sqrt(ssq[:], ssq[:])
                    rstd = ep.tile([128, 1], F32, tag="rstd")
                    nc.vector.reciprocal(rstd[:], ssq[:])
                    xc16 = ep.tile([128, D], F16, tag="xc16")
                    nc.vector.tensor_scalar(xc16[:], x[:], negmu[:], None,
                                            op0=ALU.add)
                    w16 = ep.tile([128, 1], F16, tag="w16")
                    nc.vector.tensor_scalar(w16[:], cf16[:, ut:ut + 1], rstd[:],
                                            None, op0=ALU.mult)
                    for ns, (c0, c1) in enumerate(((0, 512), (512, 768))):
                        nc.tensor.matmul(o_ps[ns][:], w16[:], xc16[:, c0:c1],
                                         start=(ut == 0), stop=(ut == VT - 1))

            o_sb = cp.tile([1, D], F32, tag="o_sb")
            nc.vector.tensor_copy(o_sb[:, 0:512], o_ps[0][:])
            nc.scalar.activation(o_sb[:, 512:768], o_ps[1][:], AF.Copy)
            nc.sync.dma_start(o_out[:], o_sb[:])

    nc.finalize()
    return nc


def _pad_wd(Wd, bd):
    """[PADK+1, 768] fp16, rows permuted to the ctx block layout
    [dims 0:32 | Z slot | dims 32:48 | 15 pad] per 64-row head block; Z-slot
    and pad rows zero.  Row PADK = bd."""
    WdT = np.asarray(Wd, np.float32).T
    out = np.zeros((PADK + 1, D), np.float16)
    for h in range(H):
        out[h * 64:h * 64 + 32, :] = WdT[h * DH:h * DH + 32, :]
        out[h * 64 + 33:h * 64 + 49, :] = WdT[h * DH + 32:(h + 1) * DH, :]
    out[PADK, :] = np.asarray(bd, np.float32)
    return out


def _core_inputs(h_b, fr, to, W):
    f16, f32 = np.float16, np.float32
    f8 = mybir.dt.np(F8)
    h = np.asarray(h_b, f32)
    hT = np.ascontiguousarray(h.T)
    fr = np.asarray(fr, np.int64) % SEQ
    to = np.asarray(to, np.int64) % SEQ
    ct = np.bincount(to, minlength=SEQ).astype(f32)
    cf = np.bincount(fr, minlength=SEQ).astype(f32)
    lnct = np.where(ct > 0, np.log(np.maximum(ct, 1e-6)), -30.0).astype(f32)
    return {
        "hT8": hT.astype(f8),
        "h_nm": np.ascontiguousarray(h).astype(f16),
        "Wq8": np.ascontiguousarray(np.asarray(W["Wq"], f32).T * WS).astype(f8),
        "Wk8": np.ascontiguousarray(np.asarray(W["Wk"], f32).T * WS).astype(f8),
        "Wcb8": np.ascontiguousarray(np.asarray(W["Wcb"], f32).T * WS).astype(f8),
        "WvTe": np.concatenate(
            [np.asarray(W["Wv"], f32).T, np.asarray(W["bv"], f32)[None, :]],
            axis=0).astype(f16),
        "WdTe": _pad_wd(W["Wd"], W["bd"]),
        "mixT": np.ascontiguousarray(np.asarray(W["mix"], f32).T),
        "lnct": np.ascontiguousarray(lnct.reshape(VT, 128).T),
        "cf16": np.ascontiguousarray(cf.reshape(VT, 128).T).astype(f16),
        "sel": _SEL,
    }


def _mk_sel():
    s = np.zeros((40, 8, 128), np.float16)
    for kk in range(8):
        base, kl = (0, kk) if kk < 4 else (32, kk - 4)
        s[base + 2 * kl, kk, 0:64] = 1.0
        s[base + 2 * kl + 1, kk, 64:128] = 1.0
    return np.ascontiguousarray(s.reshape(40, 8 * 128))


_SEL = _mk_sel()


def kernel(**inputs):
    hs = np.asarray(inputs["hidden_states"], dtype=np.float32)
    fpos = np.asarray(inputs["fpos"], dtype=np.int32)
    tpos = np.asarray(inputs["tpos"], dtype=np.int32)
    Wsets = {}
    for p in ("qtoc", "ctoq"):
        Wsets[p] = {n: np.asarray(inputs[p + "_" + n]) for n in
                    ("Wq", "Wk", "Wcb", "Wv", "Wd", "mix", "bv", "bd", "lng", "lnb")}

    tasks = [(b, d) for b in range(3) for d in ("qtoc", "ctoq")]
    tasks += [tasks[0], tasks[1]]
    in_maps = []
    for (b, d) in tasks:
        fr, to = (fpos[b], tpos[b]) if d == "qtoc" else (tpos[b], fpos[b])
        in_maps.append(_core_inputs(hs[b], fr, to, Wsets[d]))

    if "nc" not in _NC_CACHE:
        _NC_CACHE["nc"] = build_nc()
    nc = _NC_CACHE["nc"]
    res = run_bass_kernel_spmd(nc, in_maps, list(range(8)))
    results = res.results

    out = np.empty((3, 2 * D), np.float32)
    for c, (b, d) in enumerate(tasks[:6]):
        W = Wsets[d]
        p = results[c]["o"].reshape(D).astype(np.float32)
        y = np.asarray(W["lng"], np.float32) * (p / NE) + np.asarray(W["lnb"], np.float32)
        half = 0 if d == "qtoc" else 1
        out[b, half * D:(half + 1) * D] = y
    return out


if __name__ == "__main__":
    import reference
    inp = reference.setup_inputs()
    got = kernel(**{k: np.asarray(v) for k, v in inp.items()})
    exp = np.asarray(reference.reference(**inp))
    print("rel err:", np.abs(got - exp).max() / np.abs(exp).max())


# revision 48
# speedup vs baseline: 1.0652x; 1.0626x over previous
"""Trainium2 Bass kernel for nn_Encoder_79096117723504 (gnn_message_passing).

Node-space collapse (see kernel_v0): every edge-level quantity is a gather of
a node-level one, so the [H,F,T] edge attention reduces to 512-node space with
histogram weights.  This version restructures the device program around
measured TRN2 engine costs:

  * fp8e4 DoubleRow matmuls (216 ns per 256-contraction x 512-free MM, 2x the
    fp16 rate) for the score path: q/k projections and all 16 per-head
    [512,512] score matmuls.  Host pre-scales Wq/Wk/Wcb by 64 so fp8 operand
    magnitudes sit in e4m3's normal range; the 1/4096 comes out in the exp
    scale.  The v/ctx/Wd path stays fp16 (fp8 there costs 6-8e-3 rel err).
  * histogram counts ct/cf computed on HOST (np.bincount); ln(ct) is folded
    into the exp bias (per-partition in the S^T layout), which also turns the
    softmax Z row into a plain ones-contraction and drops the ct multiplies.
  * ONE batched [16,512] reciprocal for all 16 heads' softmax normalizers
    (reciprocal is ~3.3 us regardless of partition count), with Z rows
    gathered by tiny SBUF DMAs and 1/Z re-broadcast via a K=2 PE matmul.
  * LayerNorm gain/bias applied on host after pooling (exact: pooling is
    linear), with rstd folded into the pooling weight vector.
  * gpsimd unused (measured 7.5 us per [128,512] op); elementwise work split
    DVE/ACT by measured rates (ts ~330 ns, ACT copy ~686 ns, exp ~573 ns).

Per core one (segment, direction) sub-problem; cores 6,7 duplicate 0,1.
"""
import math
import os
import sys

import numpy as np

for p in ('/opt/trn_rl_repo', '/root/.axon_site/_ro/trn_rl_repo'):
    if os.path.isdir(p) and p not in sys.path:
        sys.path.insert(0, p)

import concourse.bass as bass
import concourse.mybir as mybir
from concourse import bacc, tile
from concourse.bass_utils import run_bass_kernel_spmd

F32 = mybir.dt.float32
F16 = mybir.dt.float16
F8 = mybir.dt.float8e4
AF = mybir.ActivationFunctionType
ALU = mybir.AluOpType
DR = mybir.MatmulPerfMode.DoubleRow

D = 768
H = 16
DH = 48
SEQ = 512
NE = 2048
EPS = 1e-5
SCALE = math.sqrt(D / H)
WS = 64.0                  # host weight pre-scale for the fp8 score path
EXPSC = 1.0 / (WS * WS * SCALE)   # exp scale: S_psum = WS^2 * S_true
KT = 6
VT = 4
PADK = H * 64

_NC_CACHE = {}


def build_nc():
    nc = bacc.Bacc(None, target_bir_lowering=False)

    # -------- DRAM I/O (bound by position) ---------------------------------
    hT8d = nc.declare_dram_parameter("hT8", [D, SEQ], F8, isOutput=False)
    h_nmd = nc.declare_dram_parameter("h_nm", [SEQ, D], F16, isOutput=False)
    wq8d = nc.declare_dram_parameter("Wq8", [D, D], F8, isOutput=False)
    wk8d = nc.declare_dram_parameter("Wk8", [D, D], F8, isOutput=False)
    wcb8d = nc.declare_dram_parameter("Wcb8", [D, H], F8, isOutput=False)
    wv16d = nc.declare_dram_parameter("WvTe", [D + 1, D], F16, isOutput=False)
    wd16d = nc.declare_dram_parameter("WdTe", [PADK + 1, D], F16, isOutput=False)
    mixd = nc.declare_dram_parameter("mixT", [D, H], F32, isOutput=False)
    lnctd = nc.declare_dram_parameter("lnct", [128, VT], F32, isOutput=False)
    cf16d = nc.declare_dram_parameter("cf16", [128, VT], F16, isOutput=False)
    seld = nc.declare_dram_parameter("sel", [40, 8 * 128], F16, isOutput=False)
    o_out = nc.declare_dram_parameter("o", [1, D], F32, isOutput=True)

    with tile.TileContext(nc) as tc:
        with (
            tc.tile_pool(name="const", bufs=1) as cp,
            tc.tile_pool(name="psA", bufs=3, space="PSUM") as psA,
            tc.tile_pool(name="psB", bufs=2, space="PSUM") as psB,
            tc.tile_pool(name="psC", bufs=1, space="PSUM") as psC,
            tc.tile_pool(name="psD", bufs=1, space="PSUM") as psD,
        ):
            # ---------------- bulk loads (priority order: the k/q projection
            # inputs first -- the prologue is HBM-ingress-bound) --------------
            hT8 = cp.tile([128, KT, SEQ], F8, tag="hT8")
            nc.sync.dma_start(hT8[:], hT8d[:, :].rearrange("(t p) u -> p t u", p=128))
            wk8 = cp.tile([128, KT, D], F8, tag="wk8")
            nc.sync.dma_start(wk8[:], wk8d[:, :].rearrange("(t p) d -> p t d", p=128))
            wq8 = cp.tile([128, KT, D], F8, tag="wq8")
            nc.sync.dma_start(wq8[:], wq8d[:, :].rearrange("(t p) d -> p t d", p=128))
            mix = cp.tile([128, KT, H], F32, tag="mix")
            nc.sync.dma_start(mix[:], mixd[:, :].rearrange("(t p) h -> p t h", p=128))
            wcb8 = cp.tile([128, KT, H], F8, tag="wcb8")
            nc.sync.dma_start(wcb8[:], wcb8d[:, :].rearrange("(t p) h -> p t h", p=128))
            lnct = cp.tile([128, VT], F32, tag="lnct")
            nc.sync.dma_start(lnct[:], lnctd[:])
            wv16 = cp.tile([128, KT, D], F16, tag="wv16")
            nc.sync.dma_start(wv16[:], wv16d[0:D, :].rearrange("(t p) d -> p t d", p=128))
            bv = cp.tile([1, D], F16, tag="bv")
            nc.sync.dma_start(bv[:], wv16d[D:D + 1, :])
            wd16 = cp.tile([128, 8, D], F16, tag="wd16")
            nc.sync.dma_start(wd16[:], wd16d[0:PADK, :].rearrange("(t p) d -> p t d", p=128))
            bd = cp.tile([1, D], F16, tag="bd")
            nc.sync.dma_start(bd[:], wd16d[PADK:PADK + 1, :])
            h_nm = cp.tile([128, VT, D], F16, tag="h_nm")
            nc.sync.dma_start(h_nm[:], h_nmd[:, :].rearrange("(t p) d -> p t d", p=128))
            cf16 = cp.tile([128, VT], F16, tag="cf16")
            nc.sync.dma_start(cf16[:], cf16d[:])

            ones = cp.tile([1, SEQ], F16, tag="ones")
            nc.vector.memset(ones[:], 1.0)
            # sel: per-k-tile one-hot pair-selectors; rb[m,u] = r of head
            # (2kk + (m>=64)) read straight out of rall's aligned rows
            sel = cp.tile([40, 8, 128], F16, tag="sel")
            nc.sync.dma_start(sel[:], seld[:, :].rearrange("r (k m) -> r k m", m=128))

            # PE warm-up: dummy matmuls while the bulk DMAs land, so the HAM
            # clock gate reaches K=8/8 before the first real matmul.
            warm = psD.tile([128, SEQ], F32, tag="rb", name="warm")
            for i in range(13):
                nc.tensor.matmul(warm[:], ones[:, 0:128], ones[:, :],
                                 start=True, stop=True)

            # ---------------- k/q projections (fp8 DoubleRow) ----------------
            # k first: kT16 heads the longest dependency chain (kmT -> scores)
            qT8 = cp.tile([128, KT, SEQ], F8, tag="qT8")
            kT16 = cp.tile([128, KT, SEQ], F16, tag="kT16")
            for m in range(KT):
                ps = psA.tile([128, SEQ], F32, tag="psA")
                for kp in range(3):
                    nc.tensor.matmul(ps[:], wk8[:, 2 * kp:2 * kp + 2, m * 128:(m + 1) * 128],
                                     hT8[:, 2 * kp:2 * kp + 2, :],
                                     start=(kp == 0), stop=(kp == 2), perf_mode=DR)
                nc.vector.tensor_copy(kT16[:, m, :], ps[:])
            for m in range(KT):
                ps = psA.tile([128, SEQ], F32, tag="psA")
                for kp in range(3):
                    nc.tensor.matmul(ps[:], wq8[:, 2 * kp:2 * kp + 2, m * 128:(m + 1) * 128],
                                     hT8[:, 2 * kp:2 * kp + 2, :],
                                     start=(kp == 0), stop=(kp == 2), perf_mode=DR)
                nc.scalar.activation(qT8[:, m, :], ps[:], AF.Copy)

            # ---------------- content bias + ln(ct) fold ----------------
            # cbs[v,t,h] = cb[v,h]/SCALE + ln(ct[v]); per-partition exp bias.
            cbs = cp.tile([128, VT, H], F32, tag="cbs")
            for t in range(VT):
                ps = psA.tile([128, H], F32, tag="psA", name=f"cb{t}")
                for k in range(KT):
                    nc.tensor.matmul(ps[:], hT8[:, k, t * 128:(t + 1) * 128],
                                     wcb8[:, k, :], start=(k == 0), stop=(k == KT - 1))
                nc.vector.tensor_scalar(cbs[:, t, :], ps[:], 1.0 / (WS * SCALE),
                                        lnct[:, t:t + 1],
                                        op0=ALU.mult, op1=ALU.add)

            # ---------------- v (fp16), padded head-block layout -------------
            # 64-block per head: [v 0:32 | Z-slot=1 | v 32:48 | 15 zeros]; the
            # ones in the Z slot make psc row 32 the softmax normalizer Z
            # (ct is folded into E via the ln(ct) exp bias).
            vct = cp.tile([128, VT, PADK], F8, tag="vct")
            for t in range(VT):
                blk0 = vct[:, t, :].rearrange("p (h c) -> p h c", c=64)
                nc.vector.memset(blk0[:, :, 49:64], 0.0)   # pad cols only
                nc.vector.memset(blk0[:, :, 32:33], 1.0)   # softmax-Z ones slot
                for ns, (c0, c1) in enumerate(((0, 384), (384, 768))):
                    ps = psA.tile([128, 384], F32, tag="psA")
                    for k in range(KT):
                        nc.tensor.matmul(ps[:], hT8[:, k, t * 128:(t + 1) * 128],
                                         wv16[:, k, c0:c1], start=(k == 0), stop=False)
                    nc.tensor.matmul(ps[:], ones[:, t * 128:(t + 1) * 128],
                                     bv[:, c0:c1], start=False, stop=True)
                    blk = vct[:, t, 64 * 8 * ns:64 * 8 * (ns + 1)].rearrange(
                        "p (h c) -> p h c", c=64)
                    src = ps[:, :].rearrange("p (h c) -> p h c", c=DH)
                    nc.vector.tensor_copy(blk[:, :, 0:32], src[:, :, 0:32])
                    nc.vector.tensor_copy(blk[:, :, 33:49], src[:, :, 32:DH])

            # ---------------- attention heads ----------------
            ctxU = cp.tile([128, 8, SEQ], F16, tag="ctxU")   # unnormalized ctx
            ctxT = cp.tile([128, 8, SEQ], F16, tag="ctxT")   # normalized
            # Z rows: heads 0-7 at partitions 0-7, heads 8-15 at 32-39 (engine
            # ops need 32-aligned partition bases per batch)
            Zall = cp.tile([40, SEQ], F16, tag="Zall")
            lnZ = cp.tile([40, SEQ], F32, tag="lnZ")
            rall = cp.tile([40, SEQ], F16, tag="rall")

            def zrow(hh):
                return hh if hh < 8 else 24 + hh

            def normalize_half(nb):
                """Batch-normalize head pairs 4*nb..4*nb+3: one batched
                [8,512] DVE reciprocal (cost is free-dim-bound, so batching is
                8x cheaper than per-head), then per k-tile a K=8 one-hot
                matmul replicates the two heads' 1/Z rows across the 128
                partitions, and an f16 multiply normalizes ctx."""
                r0 = 32 * nb
                if nb == 0:
                    # mid-loop: DVE reciprocal (no activation-table switch)
                    with nc.allow_low_precision(reason="softmax 1/Z in f16"):
                        nc.vector.reciprocal(rall[r0:r0 + 8, :], Zall[r0:r0 + 8, :])
                else:
                    # tail: ACT is idle; 1/Z = exp(-ln Z) -- two table-based
                    # ops instead of a 3.3us free-dim-bound DVE reciprocal,
                    # and the first table load overlaps the Z-gather DMAs
                    nc.scalar.activation(lnZ[r0:r0 + 8, :], Zall[r0:r0 + 8, :],
                                         AF.Ln)
                    nc.scalar.activation(rall[r0:r0 + 8, :], lnZ[r0:r0 + 8, :],
                                         AF.Exp, scale=-1.0)
                for kk in range(4 * nb, 4 * nb + 4):
                    rb = psD.tile([128, SEQ], F32, tag="rb", name=f"rb{kk}")
                    nc.tensor.matmul(rb[:], sel[r0:r0 + 8, kk, :],
                                     rall[r0:r0 + 8, :], start=True, stop=True)
                    nc.vector.tensor_mul(ctxT[:, kk, :], ctxU[:, kk, :], rb[:])

            with tc.tile_pool(name="head", bufs=2) as hp:
                for hpi in range(8):
                    # DoubleRow matmuls need dst partition base 0: one [64,512]
                    # PSUM tile per head (separate banks), cast into the packed
                    # [128,...] ctxU layout afterwards.
                    pscs = [psB.tile([64, SEQ], F32, tag="psc_e", bufs=1,
                                     name=f"psce{hpi}"),
                            psB.tile([64, SEQ], F32, tag="psc_o", bufs=1,
                                     name=f"psco{hpi}")]
                    Es = []
                    for sub in range(2):
                        hh = 2 * hpi + sub
                        kmT = hp.tile([128, KT, SEQ], F8, tag="kmT", bufs=3)
                        for k in range(KT):
                            nc.vector.tensor_scalar(kmT[:, k, :], kT16[:, k, :],
                                                    mix[:, k, hh:hh + 1], None,
                                                    op0=ALU.mult)
                        E8 = hp.tile([128, VT, SEQ], F8, tag="E", bufs=3)
                        for t in range(VT):
                            ps = psA.tile([128, SEQ], F32, tag="psA")
                            for kp in range(3):
                                nc.tensor.matmul(
                                    ps[:], kmT[:, 2 * kp:2 * kp + 2, t * 128:(t + 1) * 128],
                                    qT8[:, 2 * kp:2 * kp + 2, :],
                                    start=(kp == 0), stop=(kp == 2), perf_mode=DR)
                            nc.scalar.activation(E8[:, t, :], ps[:], AF.Exp,
                                                 bias=cbs[:, t, hh:hh + 1], scale=EXPSC)
                        Es.append(E8)
                    if hpi == 4:
                        normalize_half(0)   # overlaps pairs 5-7
                    # ctx matmuls after BOTH subs' scores: exp latency hidden
                    for sub in range(2):
                        hh = 2 * hpi + sub
                        for tp in range(2):
                            nc.tensor.matmul(pscs[sub][:, :],
                                             vct[:, 2 * tp:2 * tp + 2, hh * 64:(hh + 1) * 64],
                                             Es[sub][:, 2 * tp:2 * tp + 2, :],
                                             start=(tp == 0), stop=(tp == 1),
                                             perf_mode=DR)
                    # copy Z rows (PSUM row 32 of each sub) to aligned slots;
                    # single-row DVE/ACT ops are free-dim-bound (~0.5us each),
                    # so copy them as part of the full psc casts instead:
                    nc.vector.tensor_copy(ctxU[0:64, hpi, :], pscs[0][:])
                    nc.scalar.activation(ctxU[64:128, hpi, :], pscs[1][:], AF.Copy)
                    za, zb = zrow(2 * hpi), zrow(2 * hpi + 1)
                    nc.sync.dma_start(Zall[za:za + 1, :], ctxU[32:33, hpi, :])
                    nc.sync.dma_start(Zall[zb:zb + 1, :], ctxU[96:97, hpi, :])
            # Early Wd partials over the already-normalized k-tiles 0-3, plus
            # dummy matmuls gated on the last pair's ctx, keep the PE busy
            # (and the HAM clock-gate warm) through the serial tail
            # normalization chain for heads 8-15.
            wd_open = {}
            for (ut, ns, c0, c1) in ((0, 0, 0, 512), (0, 1, 512, 768),
                                     (1, 0, 0, 512)):
                ps = psA.tile([128, c1 - c0], F32, tag="psA", name=f"wde{ut}{ns}")
                for k in range(4):
                    nc.tensor.matmul(ps[:], ctxT[:, k, ut * 128:(ut + 1) * 128],
                                     wd16[:, k, c0:c1], start=(k == 0), stop=False)
                wd_open[(ut, ns)] = ps

            normalize_half(1)

            # ---------------- epilogue: Wd, residual, LN, pooled mean --------
            o_ps = [psC.tile([1, 512], F32, tag="ops0", name="ops0"),
                    psC.tile([1, 256], F32, tag="ops1", name="ops1")]
            with tc.tile_pool(name="epi", bufs=2) as ep:
                for ut in range(VT):
                    x = ep.tile([128, D], F32, tag="x")
                    for ns, (c0, c1) in enumerate(((0, 512), (512, 768))):
                        ps = wd_open.pop((ut, ns), None)
                        if ps is None:
                            ps = psA.tile([128, c1 - c0], F32, tag="psA")
                            k0 = 0
                        else:
                            k0 = 4
                        for k in range(k0, 8):
                            nc.tensor.matmul(ps[:], ctxT[:, k, ut * 128:(ut + 1) * 128],
                                             wd16[:, k, c0:c1], start=(k == 0), stop=False)
                        nc.tensor.matmul(ps[:], ones[:, ut * 128:(ut + 1) * 128],
                                         bd[:, c0:c1], start=False, stop=True)
                        nc.vector.tensor_add(x[:, c0:c1], ps[:], h_nm[:, ut, c0:c1])
                    negmu = ep.tile([128, 1], F32, tag="negmu")
                    nc.vector.reduce_sum(negmu[:], x[:], axis=mybir.AxisListType.X)
                    nc.vector.tensor_scalar(negmu[:], negmu[:], -1.0 / D, None,
                                            op0=ALU.mult)
                    sq = ep.tile([128, D], F16, tag="sq")
                    ssq = ep.tile([128, 1], F32, tag="ssq")
                    nc.scalar.activation(sq[:], x[:], AF.Square,
                                         bias=negmu[:], scale=1.0, accum_out=ssq[:])
                    nc.vector.tensor_scalar(ssq[:], ssq[:], 1.0 / D, EPS,
                                            op0=ALU.mult, op1=ALU.add)
                    nc.scalar.sqrt(ssq[:], ssq[:])
                    rstd = ep.tile([128, 1], F32, tag="rstd")
                    nc.vector.reciprocal(rstd[:], ssq[:])
                    xc16 = ep.tile([128, D], F16, tag="xc16")
                    nc.vector.tensor_scalar(xc16[:], x[:], negmu[:], None,
                                            op0=ALU.add)
                    w16 = ep.tile([128, 1], F16, tag="w16")
                    nc.vector.tensor_scalar(w16[:], cf16[:, ut:ut + 1], rstd[:],
                                            None, op0=ALU.mult)
                    for ns, (c0, c1) in enumerate(((0, 512), (512, 768))):
                        nc.tensor.matmul(o_ps[ns][:], w16[:], xc16[:, c0:c1],
                                         start=(ut == 0), stop=(ut == VT - 1))

            o_sb = cp.tile([1, D], F32, tag="o_sb")
            nc.vector.tensor_copy(o_sb[:, 0:512], o_ps[0][:])
            nc.scalar.activation(o_sb[:, 512:768], o_ps[1][:], AF.Copy)
            nc.sync.dma_start(o_out[:], o_sb[:])

    nc.finalize()
    return nc


def _pad_wd(Wd, bd):
    """[PADK+1, 768] fp16, rows permuted to the ctx block layout
    [dims 0:32 | Z slot | dims 32:48 | 15 pad] per 64-row head block; Z-slot
    and pad rows zero.  Row PADK = bd."""
    WdT = np.asarray(Wd, np.float32).T
    out = np.zeros((PADK + 1, D), np.float16)
    for h in range(H):
        out[h * 64:h * 64 + 32, :] = WdT[h * DH:h * DH + 32, :]
        out[h * 64 + 33:h * 64 + 49, :] = WdT[h * DH + 32:(h + 1) * DH, :]
    out[PADK, :] = np.asarray(bd, np.float32)
    return out


def _core_inputs(h_b, fr, to, W):
    f16, f32 = np.float16, np.float32
    f8 = mybir.dt.np(F8)
    h = np.asarray(h_b, f32)
    hT = np.ascontiguousarray(h.T)
    fr = np.asarray(fr, np.int64) % SEQ
    to = np.asarray(to, np.int64) % SEQ
    ct = np.bincount(to, minlength=SEQ).astype(f32)
    cf = np.bincount(fr, minlength=SEQ).astype(f32)
    lnct = np.where(ct > 0, np.log(np.maximum(ct, 1e-6)), -30.0).astype(f32)
    return {
        "hT8": hT.astype(f8),
        "h_nm": np.ascontiguousarray(h).astype(f16),
        "Wq8": np.ascontiguousarray(np.asarray(W["Wq"], f32).T * WS).astype(f8),
        "Wk8": np.ascontiguousarray(np.asarray(W["Wk"], f32).T * WS).astype(f8),
        "Wcb8": np.ascontiguousarray(np.asarray(W["Wcb"], f32).T * WS).astype(f8),
        "WvTe": np.concatenate(
            [np.asarray(W["Wv"], f32).T, np.asarray(W["bv"], f32)[None, :]],
            axis=0).astype(f16),
        "WdTe": _pad_wd(W["Wd"], W["bd"]),
        "mixT": np.ascontiguousarray(np.asarray(W["mix"], f32).T),
        "lnct": np.ascontiguousarray(lnct.reshape(VT, 128).T),
        "cf16": np.ascontiguousarray(cf.reshape(VT, 128).T).astype(f16),
        "sel": _SEL,
    }


def _mk_sel():
    s = np.zeros((40, 8, 128), np.float16)
    for kk in range(8):
        base, kl = (0, kk) if kk < 4 else (32, kk - 4)
        s[base + 2 * kl, kk, 0:64] = 1.0
        s[base + 2 * kl + 1, kk, 64:128] = 1.0
    return np.ascontiguousarray(s.reshape(40, 8 * 128))


_SEL = _mk_sel()


def kernel(**inputs):
    hs = np.asarray(inputs["hidden_states"], dtype=np.float32)
    fpos = np.asarray(inputs["fpos"], dtype=np.int32)
    tpos = np.asarray(inputs["tpos"], dtype=np.int32)
    Wsets = {}
    for p in ("qtoc", "ctoq"):
        Wsets[p] = {n: np.asarray(inputs[p + "_" + n]) for n in
                    ("Wq", "Wk", "Wcb", "Wv", "Wd", "mix", "bv", "bd", "lng", "lnb")}

    tasks = [(b, d) for b in range(3) for d in ("qtoc", "ctoq")]
    tasks += [tasks[0], tasks[1]]
    in_maps = []
    for (b, d) in tasks:
        fr, to = (fpos[b], tpos[b]) if d == "qtoc" else (tpos[b], fpos[b])
        in_maps.append(_core_inputs(hs[b], fr, to, Wsets[d]))

    if "nc" not in _NC_CACHE:
        _NC_CACHE["nc"] = build_nc()
    nc = _NC_CACHE["nc"]
    res = run_bass_kernel_spmd(nc, in_maps, list(range(8)))
    results = res.results

    out = np.empty((3, 2 * D), np.float32)
    for c, (b, d) in enumerate(tasks[:6]):
        W = Wsets[d]
        p = results[c]["o"].reshape(D).astype(np.float32)
        y = np.asarray(W["lng"], np.float32) * (p / NE) + np.asarray(W["lnb"], np.float32)
        half = 0 if d == "qtoc" else 1
        out[b, half * D:(half + 1) * D] = y
    return out


if __name__ == "__main__":
    import reference
    inp = reference.setup_inputs()
    got = kernel(**{k: np.asarray(v) for k, v in inp.items()})
    exp = np.asarray(reference.reference(**inp))
    print("rel err:", np.abs(got - exp).max() / np.abs(exp).max())
